# revision 4
# baseline (speedup 1.0000x reference)
"""Trainium2 Bass kernel for NeuronInvariantDeepSetLayer (segment_reduce).

kernel(**inputs) takes FULL unsharded inputs (as in reference.setup_inputs())
and returns the full [4096, 1] float32 output.

Strategy: data-parallel over 8 NeuronCores. Segments are split 512/core
(idx is sorted, so each core's rows are a contiguous slice of x). Rows are
host-padded so that each 128-segment block starts exactly at a 128-row tile
boundary -> every core runs the IDENTICAL instruction stream (pure SPMD),
only the data differs.

Key layout decision: x is pre-transposed PER 128-ROW TILE on the host and
uploaded as bf16. Each SBUF tile [128 part, 128] then holds xT directly
(din-chunk on partitions), so it can be fed to the PE as the stationary
operand with NO on-chip transposes, and HBM traffic is halved vs f32.

Algebraic restructure vs the straight reference: segment-sum commutes with
the (linear) second phi layer, so we compute
    xsum1 = segsum(relu(x @ W1 + b1));  xsum2 = xsum1 @ W2 + count*b2
and only apply W2 on the [512, 192] per-core segment sums (4 blocks of 128).
This removes the per-row mm2 entirely.

Per core device pipeline:
  - DMA: xT tiles bf16 HBM -> SBUF, 1024 rows (= 8 tiles) per DMA
  - mm1: for each 128-row tile: 6x matmul(lhsT=xT[kc], rhs=W1[kc]) -> h1
    psum [128 rows, 192] (f32 accum)
  - ACT relu psum -> SBUF bf16 h1b
  - sel = is_equal(idx_local, iota) one-hot [128 rows, 128 segs] (DVE)
  - seg reduce: matmul(pseg[blk] += sel.T @ h1b) accumulated in PSUM
  - rho tail per 128-seg block: transpose xsum1 via identity matmuls,
    then W2, rho_w1+relu, rho_w2 -> out [128] per block
"""

import sys
import os

sys.path.insert(0, "/opt/trn_rl_repo")

import numpy as np
import ml_dtypes

N = 400000
B = 4096
DIN = 768
DHID = 192
NCORES = 8
SPC = B // NCORES  # segments per core = 512
SBLK = 128  # segments per seg-block (psum accumulator width)
NBLK = SPC // SBLK  # 4 seg-blocks per core
P = 128
KC1 = DIN // P  # 6 k-chunks for mm1
CH = 1024  # rows per DMA chunk (8 tiles)
TPC = CH // P  # tiles per chunk = 8

f32 = np.float32
bf16 = ml_dtypes.bfloat16

# walrus --enable-ldw-opt=true (bass default false). Safe here: this kernel
# emits no transpose-mode ldweights (which that pass can't handle).
USE_LDW_OPT = True


def _prep(x, idx):
    """Host-side sharding + per-tile transpose + bf16 cast.

    Returns xs_t[c]: [nchunks, 128, TPC*KC1*128] bf16 where
    xs_t[c][ch][p][(n*KC1+kc)*128+r] = x_row(c, ch*CH + n*128 + r)[kc*128+p],
    ixs_arr[c]: [nchunks, 128, TPC] f32 local segment ids (1e9 padding),
    plus tblk and segment counts.
    """
    if np.any(np.diff(idx) < 0):  # defensive: spec says idx is sorted
        order = np.argsort(idx, kind="stable")
        x, idx = x[order], idx[order]
    counts = np.bincount(idx, minlength=B)
    assert counts.sum() == x.shape[0]
    bounds = np.concatenate([[0], np.cumsum(counts)]).astype(np.int64)
    blk_rows = counts.reshape(NCORES * NBLK, SBLK).sum(1)
    tblk = int(np.ceil(blk_rows.max() / P))
    tblk = ((tblk + 3) // 4) * 4  # multiple of 4 -> NP % 1024 == 0
    NP = NBLK * tblk * P
    nchunks = NP // CH
    x16 = x.astype(bf16)
    xs_t = np.zeros((NCORES, nchunks, P, TPC, KC1, P), bf16)
    ixs = np.full((NCORES, NP), 1.0e9, f32)
    for c in range(NCORES):
        xc = np.zeros((NP, DIN), bf16)
        for blk in range(NBLK):
            s0 = c * SPC + blk * SBLK
            r0, r1 = int(bounds[s0]), int(bounds[s0 + SBLK])
            d0 = blk * tblk * P
            xc[d0 : d0 + (r1 - r0)] = x16[r0:r1]
            ixs[c, d0 : d0 + (r1 - r0)] = (idx[r0:r1] - c * SPC).astype(f32)
        # [ch, n, r, kc, p] -> [ch, p, n, kc, r]
        xs_t[c] = xc.reshape(nchunks, TPC, P, KC1, P).transpose(0, 4, 1, 3, 2)
    xs_t = xs_t.reshape(NCORES, nchunks, P, TPC * KC1 * P)
    # pre-arrange idx so each partition's DMA read is contiguous:
    # ixs_arr[c, ch, p, n] = ixs[c, ch*CH + n*P + p]
    ixs_arr = np.ascontiguousarray(
        ixs.reshape(NCORES, nchunks, TPC, P).transpose(0, 1, 3, 2)
    )
    return xs_t, ixs_arr, tblk, counts


def _build(tblk, phi_w1, phi_b1, phi_w2, phi_b2, rho_w1, rho_b1, rho_w2, rho_b2):
    import concourse.bacc as bacc
    import concourse.mybir as mybir
    import concourse.tile as tile

    BF = mybir.dt.bfloat16
    F32 = mybir.dt.float32
    Relu = mybir.ActivationFunctionType.Relu
    Copy = mybir.ActivationFunctionType.Copy

    has_b1 = bool(np.any(phi_b1 != 0))
    has_b2 = bool(np.any(phi_b2 != 0))
    has_rb1 = bool(np.any(rho_b1 != 0))
    has_rb2 = bool(np.any(rho_b2 != 0))
    HW = DHID + 1 if has_b2 else DHID  # h1b/pseg width (count col when b2!=0)

    # ---- packed constants (inlined into the NEFF) ----
    # w1r[p, kc, h] = W1[kc*128+p, h]
    w1r = np.ascontiguousarray(
        phi_w1.reshape(KC1, P, DHID).transpose(1, 0, 2)
    ).astype(bf16)
    w2c0 = np.ascontiguousarray(phi_w2[0:P, :]).astype(bf16)  # [128, 192]
    w2c1 = np.ascontiguousarray(phi_w2[P:DHID, :]).astype(bf16)  # [64, 192]
    rw1k = np.ascontiguousarray(rho_w1.reshape(2, 96, 6).transpose(1, 0, 2)).astype(f32)
    rw2k = np.ascontiguousarray(rho_w2).astype(f32)  # [6, 1]
    idn16 = np.eye(P, dtype=bf16)
    jmat = np.ascontiguousarray(
        np.broadcast_to(
            (np.arange(NBLK)[:, None] * SBLK + np.arange(SBLK)[None, :]).astype(f32),
            (P, NBLK, SBLK),
        )
    )
    b1row = np.ascontiguousarray(phi_b1.reshape(1, DHID)).astype(f32)
    ones1 = np.ones((1, P), f32)
    onesc = np.ones((P, 1), bf16)
    b2row = np.ascontiguousarray(phi_b2.reshape(1, DHID)).astype(bf16)
    rb1k = np.ascontiguousarray(rho_b1.reshape(6, 1)).astype(f32)
    rb2k = np.ascontiguousarray(rho_b2.reshape(1, 1)).astype(f32)

    NP = NBLK * tblk * P
    nchunks = NP // CH
    CW = TPC * KC1 * P  # per-partition bf16 elems per chunk

    nc = bacc.Bacc(None, target_bir_lowering=False)
    x_in = nc.dram_tensor("xt_shard", [nchunks, P, CW], BF, kind="ExternalInput")
    ix_in = nc.dram_tensor("idxlf", [nchunks, P, TPC], F32, kind="ExternalInput")
    out_d = nc.dram_tensor("out_shard", [SPC], F32, kind="ExternalOutput")

    w1d = nc.inline_tensor(w1r, "w1r")
    w2d0 = nc.inline_tensor(w2c0, "w2c0")
    w2d1 = nc.inline_tensor(w2c1, "w2c1")
    rw1d = nc.inline_tensor(rw1k, "rw1k")
    rw2d = nc.inline_tensor(rw2k, "rw2k")
    idn16d = nc.inline_tensor(idn16, "idn16")
    jmatd = nc.inline_tensor(jmat, "jmat")
    b1d = nc.inline_tensor(b1row, "b1row") if has_b1 else None
    ones1d = nc.inline_tensor(ones1, "ones1") if has_b1 else None
    onescd = nc.inline_tensor(onesc, "onesc") if has_b2 else None
    b2d = nc.inline_tensor(b2row, "b2row") if has_b2 else None
    rb1d = nc.inline_tensor(rb1k, "rb1k") if has_rb1 else None
    rb2d = nc.inline_tensor(rb2k, "rb2k") if has_rb2 else None

    with tile.TileContext(nc) as tc:
        with (
            tc.tile_pool(name="consts", bufs=1) as cpool,
            tc.tile_pool(name="xb", bufs=5) as xpool,
            tc.tile_pool(name="ixb", bufs=5) as ixpool,
            tc.tile_pool(name="h1b", bufs=4) as h1pool,
            tc.tile_pool(name="selb", bufs=4) as selpool,
            tc.tile_pool(name="rho", bufs=1) as rhopool,
            tc.tile_pool(name="ph1", bufs=3, space="PSUM") as ph1,
            tc.tile_pool(name="pseg", bufs=2, space="PSUM") as pseg,
            tc.tile_pool(name="prho", bufs=1, space="PSUM") as prho,
        ):
            # ---- load constants into SBUF ----
            w1s = cpool.tile_from(w1d[:])
            w2s0 = cpool.tile_from(w2d0[:])
            w2s1 = cpool.tile_from(w2d1[:])
            rw1s = cpool.tile_from(rw1d[:])
            rw2s = cpool.tile_from(rw2d[:])
            idn16s = cpool.tile_from(idn16d[:])
            js = cpool.tile_from(jmatd[:])
            b1s = cpool.tile_from(b1d[:]) if has_b1 else None
            ones1s = cpool.tile_from(ones1d[:]) if has_b1 else None
            onescs = cpool.tile_from(onescd[:]) if has_b2 else None
            b2s = cpool.tile_from(b2d[:]) if has_b2 else None
            rb1s = cpool.tile_from(rb1d[:]) if has_rb1 else None
            rb2s = cpool.tile_from(rb2d[:]) if has_rb2 else None

            pseg_tiles = {}
            pending = []  # deferred seg matmuls: (selb, h1b, t)

            def emit_seg(selb_t, h1b_t, t):
                blk = t // tblk
                if t % tblk == 0:
                    pseg_tiles[blk] = pseg.tile(
                        [P, HW], F32, tag="seg", name=f"pseg_{blk}"
                    )
                nc.tensor.matmul(
                    out=pseg_tiles[blk][:],
                    lhsT=selb_t[:],
                    rhs=h1b_t[:],
                    start=(t % tblk == 0),
                    stop=(t % tblk == tblk - 1),
                )
                if t % tblk == tblk - 1:
                    emit_rho(blk, pseg_tiles.pop(blk))

            def emit_rho(blk, pseg_t):
                # xsum1 [128 segs, HW] f32 psum -> out[blk*128:(blk+1)*128]
                xsb = rhopool.tile([P, HW], BF, tag="xsb")
                nc.scalar.copy(out=xsb[:], in_=pseg_t[:])
                # transpose via identity matmul (NOT transpose-mode; ldw-opt ok)
                pxT = prho.tile([P, 2, P], F32, tag="rxt", name=f"pxT_{blk}")
                nc.tensor.matmul(
                    out=pxT[:, 0, :], lhsT=xsb[:, 0:P], rhs=idn16s[:],
                    start=True, stop=True,
                )
                nc.tensor.matmul(
                    out=pxT[0 : HW - P, 1, :], lhsT=xsb[:, P:HW], rhs=idn16s[:],
                    start=True, stop=True,
                )
                xsT = rhopool.tile([P, 2, P], BF, tag="xsT")
                nc.vector.tensor_copy(out=xsT[:, 0, :], in_=pxT[:, 0, :])
                nc.vector.tensor_copy(
                    out=xsT[0 : HW - P, 1, :], in_=pxT[0 : HW - P, 1, :]
                )
                # xsum2T [192 hid2 (2x96), 128 segs] = W2.T @ xsum1T (+ b2*cnt)
                p2T = prho.tile([96, 2, P], F32, tag="r2t", name=f"p2T_{blk}")
                for mc in range(2):
                    nc.tensor.matmul(
                        out=p2T[:, mc, :],
                        lhsT=w2s0[:, mc * 96 : (mc + 1) * 96],
                        rhs=xsT[:, 0, :],
                        start=True, stop=False,
                    )
                    nc.tensor.matmul(
                        out=p2T[:, mc, :],
                        lhsT=w2s1[:, mc * 96 : (mc + 1) * 96],
                        rhs=xsT[0:64, 1, :],
                        start=False, stop=not has_b2,
                    )
                    if has_b2:
                        nc.tensor.matmul(
                            out=p2T[:, mc, :],
                            lhsT=b2s[0:1, mc * 96 : (mc + 1) * 96],
                            rhs=xsT[64:65, 1, :],
                            start=False, stop=True,
                        )
                x2b = rhopool.tile([96, 2, P], F32, tag="x2b")
                nc.vector.tensor_copy(out=x2b[:], in_=p2T[:])
                # r [6, 128 segs] = relu(rho_w1.T @ xsum2T + rb1)
                prT = prho.tile([6, P], F32, tag="rsm", name=f"prT_{blk}")
                for mc in range(2):
                    nc.tensor.matmul(
                        out=prT[:],
                        lhsT=rw1s[:, mc, :],
                        rhs=x2b[:, mc, :],
                        start=(mc == 0),
                        stop=(mc == 1),
                    )
                rtb = rhopool.tile([6, P], F32, tag="rtb")
                if has_rb1:
                    nc.scalar.activation(out=rtb[:], in_=prT[:], func=Relu, bias=rb1s[:])
                else:
                    nc.scalar.activation(out=rtb[:], in_=prT[:], func=Relu)
                pot = prho.tile([1, P], F32, tag="rsm", name=f"pot_{blk}")
                nc.tensor.matmul(out=pot[:], lhsT=rw2s[:], rhs=rtb[:], start=True, stop=True)
                ob = rhopool.tile([1, P], F32, tag="ob")
                if has_rb2:
                    nc.scalar.activation(out=ob[:], in_=pot[:], func=Copy, bias=rb2s[:])
                else:
                    nc.scalar.copy(out=ob[:], in_=pot[:])
                nc.sync.dma_start(out=out_d[blk * SBLK : (blk + 1) * SBLK], in_=ob[:])

            for ch in range(nchunks):
                if ch < 4:
                    # stream the first chunks at tile granularity on two
                    # queues so the PE never starves (and HAM stays warm)
                    # during ramp-up
                    xb0 = []
                    for q in range(TPC):
                        xq = xpool.tile(
                            [P, KC1, P], BF, tag=f"xb0_{q}", name=f"xb0_{ch}_{q}",
                            bufs=2,
                        )
                        eng = nc.gpsimd if q % 2 == 0 else nc.sync
                        eng.dma_start(
                            out=xq[:],
                            in_=x_in[ch, :, q * KC1 * P : (q + 1) * KC1 * P].rearrange(
                                "p (kc r) -> p kc r", kc=KC1
                            ),
                        )
                        xb0.append(xq)

                    def xt_at(n, kc, _tiles=xb0):
                        return _tiles[n][:, kc, :]
                else:
                    xb = xpool.tile([P, TPC, KC1, P], BF, tag="xb")
                    nc.gpsimd.dma_start(
                        out=xb[:],
                        in_=x_in[ch].rearrange("p (n kc r) -> p n kc r", n=TPC, kc=KC1),
                    )

                    def xt_at(n, kc):
                        return xb[:, n, kc, :]

                ixb = ixpool.tile([P, TPC], F32, tag="ixb")
                nc.sync.dma_start(out=ixb[:], in_=ix_in[ch])
                for n in range(TPC):
                    t = ch * TPC + n
                    blk = t // tblk
                    # mm1: h1 [128 rows, 192] = sum_kc xT[kc].T @ W1[kc]
                    ph1t = ph1.tile([P, DHID], F32, tag="h1", name=f"ph1_{t}")
                    for kc in range(KC1):
                        nc.tensor.matmul(
                            out=ph1t[:],
                            lhsT=xt_at(n, kc),
                            rhs=w1s[:, kc, :],
                            start=(kc == 0),
                            stop=(kc == KC1 - 1 and not has_b1),
                        )
                    if has_b1:
                        nc.tensor.matmul(
                            out=ph1t[:], lhsT=ones1s[:], rhs=b1s[:],
                            start=False, stop=True,
                        )
                    # defer seg matmul 2 tiles so ACT relu has time to drain
                    if len(pending) >= 2:
                        emit_seg(*pending.pop(0))
                    h1b = h1pool.tile([P, HW], BF, tag="h1b", name=f"h1b_{t}")
                    nc.scalar.activation(out=h1b[:, 0:DHID], in_=ph1t[:], func=Relu)
                    if has_b2:
                        nc.vector.tensor_copy(out=h1b[:, DHID:HW], in_=onescs[:])
                    selb = selpool.tile([P, P], BF, tag="selb", name=f"sel_{t}")
                    nc.vector.tensor_tensor(
                        out=selb[:],
                        in0=ixb[:, n : n + 1].to_broadcast([P, P]),
                        in1=js[:, blk, :],
                        op=mybir.AluOpType.is_equal,
                    )
                    pending.append((selb, h1b, t))
            while pending:
                emit_seg(*pending.pop(0))

    nc.compile()
    return nc


_CACHE = {}


def _get_nc(tblk, weights):
    key = tblk
    if key not in _CACHE:
        if USE_LDW_OPT:
            import concourse.bass_utils as bu

            orig = bu.run_command

            def run_command_ldwopt(argv, **kw):
                argv = [
                    "--enable-ldw-opt=true" if a == "--enable-ldw-opt=false" else a
                    for a in argv
                ]
                return orig(argv, **kw)

            bu.run_command = run_command_ldwopt
            try:
                _CACHE[key] = _build(tblk, *weights)
            finally:
                bu.run_command = orig
        else:
            _CACHE[key] = _build(tblk, *weights)
    return _CACHE[key]


def _run(inputs, trace=False):
    from concourse.bass_utils import run_bass_kernel_spmd

    inp = {k: np.asarray(v) for k, v in inputs.items()}
    x = inp["x"].astype(f32, copy=False)
    idx = inp["idx"].astype(np.int32, copy=False)
    weights = tuple(
        inp[k].astype(f32, copy=False)
        for k in ("phi_w1", "phi_b1", "phi_w2", "phi_b2", "rho_w1", "rho_b1", "rho_w2", "rho_b2")
    )
    xs_t, ixs, tblk, counts = _prep(x, idx)
    nc = _get_nc(tblk, weights)
    in_maps = [{"xt_shard": xs_t[c], "idxlf": ixs[c]} for c in range(NCORES)]
    res = run_bass_kernel_spmd(nc, in_maps, core_ids=list(range(NCORES)), trace=trace)
    out = np.concatenate([res.results[c]["out_shard"] for c in range(NCORES)])
    out = out.reshape(B, 1).astype(f32)
    # safety net: empty segments (never happens for the target distribution)
    if np.any(counts == 0):
        (phi_w1, phi_b1, phi_w2, phi_b2, rho_w1, rho_b1, rho_w2, rho_b2) = weights
        z = np.zeros((1, DHID), f32)
        r = np.maximum(z @ rho_w1 + rho_b1, 0.0)
        o0 = (r @ rho_w2 + rho_b2).astype(f32)
        out[counts == 0] = o0
    return out, res


def kernel(**inputs) -> np.ndarray:
    return _run(inputs, trace=False)[0]


if __name__ == "__main__":
    # quick self-test against numpy
    rng = np.random.default_rng(0)
    x = rng.standard_normal((N, DIN)).astype(f32)
    idx = np.sort(rng.integers(0, B, N).astype(np.int32))
    w1 = (rng.standard_normal((DIN, DHID)) / np.sqrt(DIN)).astype(f32)
    w2 = (rng.standard_normal((DHID, DHID)) / np.sqrt(DHID)).astype(f32)
    r1 = (rng.standard_normal((DHID, 6)) / np.sqrt(DHID)).astype(f32)
    r2 = (rng.standard_normal((6, 1)) / np.sqrt(6)).astype(f32)
    inputs = dict(
        x=x, idx=idx,
        phi_w1=w1, phi_b1=np.zeros(DHID, f32), phi_w2=w2, phi_b2=np.zeros(DHID, f32),
        rho_w1=r1, rho_b1=np.zeros(6, f32), rho_w2=r2, rho_b2=np.zeros(1, f32),
    )
    out = kernel(**inputs)
    h = np.maximum(x @ w1, 0.0) @ w2
    xsum = np.zeros((B, DHID), f32)
    np.add.at(xsum, idx, h)
    exp = np.maximum(xsum @ r1, 0.0) @ r2
    rel = np.linalg.norm(out - exp) / np.linalg.norm(exp)
    print("self-test rel err:", rel)
